# revision 40
# baseline (speedup 1.0000x reference)
"""Trainium2 Bass kernel for nn_Attention (dense_transformer, ridge regime).

Computation per batch b:
    scores[s]  = <lstm_output[b,s,:], hidden[b,:]>          # [S]
    w          = softmax(scores)                            # [S]
    attn[h]    = sum_s w[s] * lstm_output[b,s,h]            # [H]
    out[b]     = [hidden[b], attn] @ W_combine.T + b_combine

Sharding: data-parallel over batch B=64 across 8 cores (8 batches/core).

v3 design (fp16 data path; measured rel-err ~1.2e-3 vs 2e-2 gate):
  - Host converts lstm_output to fp16, partition-major [BPC, 128, T, H]:
    each batch one contiguous-per-partition 4MB DMA in 1MB quarters,
    L triple-buffered. HBM/core: 32MB lstm + 4MB W.T + 2MB hidR + small.
  - hidR (hidden replicated across partitions) comes from one broadcast
    DMA (stride-0 partition read of hidden16) -- no sel-matmul staging.
  - Scores per batch: 4 quad-multiplies on DVE (fp16 2x, one per DMA
    quarter, in1 = hidR stride-0 broadcast over the t dim), each into its
    own prodQ buffer so ACT never paces DVE. Columns 0-10 reduced by ACT
    accum-copies; 11-15 by DVE (single + quad tensor_reduce).
  - Softmax max chain via PE transposes; exp on ACT (fp16 wexp out, f32 Z
    accum); einsum2 and final projection as fp16 PE matmuls; transposes
    stay f32 into PSUM, converting on the ACT/DVE copy-out.
  - W.T fp16 [128, 16, 1024] loads mid-stream into its own SBUF buffer.
"""

import numpy as np

import concourse.bass as bass
from concourse import bass_isa, library_config, mybir
from concourse.bass_utils import run_bass_kernel_spmd

F32 = mybir.dt.float32
F16 = mybir.dt.float16

B, S, H = 64, 2048, 1024
NCORES = 8
BPC = B // NCORES          # batches per core
T = S // 128               # s-tiles per batch
NCH = (2 * H) // 128       # 16 chunks of the combined dim
HCH = H // 128             # 8 chunks of one H
NQ = 4                     # DMA quarters / mult quads per batch
TQ = T // NQ               # t-tiles per quarter
NL = 3                     # L buffer slots (triple buffer)

NACT = 12                  # score cols 0..NACT-1 on ACT; 12-15 on DVE
GPS_FOLD_QS = (1,)         # quads whose cols ACT reads GPS-pre-folded to 512
ACT_ORDER = [0, 1, 2, 3, 8, 9, 10, 11, 4, 5, 6, 7]  # fulls first, then shorts

_cached_nc = None
last_results = None


def _build_program():
    nc = bass.Bass()

    lstm_d = nc.declare_dram_parameter("lstm16", [BPC, 128, T, H], F16, isOutput=False)
    hid_d = nc.declare_dram_parameter("hidden", [BPC, H], F32, isOutput=False)
    hid16_d = nc.declare_dram_parameter("hidden16", [BPC, H], F16, isOutput=False)
    wt_d = nc.declare_dram_parameter("w_t16", [128, NCH, H], F16, isOutput=False)
    b_d = nc.declare_dram_parameter("b_combine", [H], F32, isOutput=False)
    out_d = nc.declare_dram_parameter("out", [BPC, H], F32, isOutput=True)

    # ---- SBUF ----
    L = [nc.alloc_sbuf_tensor(f"L{i}", [128, T, H], F16) for i in range(NL)]  # 3x4MB
    WT = nc.alloc_sbuf_tensor("WT", [128, NCH, H], F16)                       # 4MB
    hid_t = nc.alloc_sbuf_tensor("hid", [BPC, H], F32)
    hid = hid_t.ap()
    bias_t = nc.alloc_sbuf_tensor("bias", [BPC, H], F32)
    bias = bias_t.ap()
    out_t = nc.alloc_sbuf_tensor("out_sb", [BPC, H], F32)
    out_sb = out_t.ap()
    hidR = nc.alloc_sbuf_tensor("hidR", [128, 2, H], F16)  # rotating bcast hidden
    prodQ = [nc.alloc_sbuf_tensor(f"prodQ{i}", [128, TQ, H], F16) for i in range(NQ)]
    fold1 = [nc.alloc_sbuf_tensor(f"fold1_{i}", [128, TQ, 512], F32)
             for i in range(2)]   # parity-doubled fold buffer
    gdum = nc.alloc_sbuf_tensor("gdum", [1, 1], F32)
    dmy = nc.alloc_sbuf_tensor("dmy", [128, T], F32)
    CT = nc.alloc_sbuf_tensor("CT", [128, NCH, BPC], F16)     # combined^T
    scores = [nc.alloc_sbuf_tensor(f"scores{i}", [128, T], F32) for i in range(2)]
    wexp = [nc.alloc_sbuf_tensor(f"wexp{i}", [128, T], F16) for i in range(2)]
    zp = [nc.alloc_sbuf_tensor(f"zp{i}", [128, 1], F32) for i in range(2)]
    mp = nc.alloc_sbuf_tensor("mp", [128, 1], F32)
    negM1_t = nc.alloc_sbuf_tensor("negM1s", [1, 2], F32)
    negM1 = [negM1_t.ap()[0:1, i:i + 1] for i in range(2)]
    negM = [nc.alloc_sbuf_tensor(f"negM{i}", [128, 1], F32) for i in range(2)]
    rZ_t = nc.alloc_sbuf_tensor("rZs", [1, 2], F32)
    rZ = [rZ_t.ap()[0:1, i:i + 1] for i in range(2)]
    ones128 = nc.alloc_sbuf_tensor("ones128", [128, 1], F32)
    attn2 = nc.alloc_sbuf_tensor("attn2", [1, 2 * H], F32)
    attn_sb = [attn2.ap()[0:1, i * H:(i + 1) * H] for i in range(2)]
    ones_col = nc.alloc_sbuf_tensor("ones_col", [1, 128], F32)
    ident = nc.alloc_sbuf_tensor("ident", [128, 128], F32)

    # ---- PSUM ----
    acc_lo = nc.alloc_psum_tensor("acc_lo", [BPC, 512], F32)  # einsum2 row 0 / final
    acc_hi = nc.alloc_psum_tensor("acc_hi", [BPC, 512], F32)
    ct8_t = nc.alloc_psum_tensor("ct8", [128, HCH, BPC], F32) # hidT staging
    ctc_t = nc.alloc_psum_tensor("ctc", [128, 512], F32)      # attnT transposes
    mpT_t = nc.alloc_psum_tensor("mpT", [1, 128], F32)        # transp target
    negM_t = nc.alloc_psum_tensor("negMbc", [128, 1], F32)    # bcast mm target
    Zps_t = nc.alloc_psum_tensor("Zps", [1, 1], F32)          # Z mm target
    mpT = mpT_t.ap()
    negM_bc = negM_t.ap()
    Zps = Zps_t.ap()
    ctcols8 = ct8_t.ap()
    ctcols = ctc_t.ap()[:, 0:HCH]

    # ---------------- two-pass emission ----------------
    ev = {}
    sems = {}
    counts = {}

    class Prog:
        def __init__(self, name):
            self.name = name
            self.emit = False
            self.eng = None
            self.hwm = {}
            self.auto_drain = name in ("dve", "act", "gps")
            self.first_op = True

        def begin(self, eng=None, emit=False):
            self.emit = emit
            self.eng = eng
            self.hwm = {}
            self.first_op = True

        def wait(self, key):
            """key: event tuple, or (sem_name, value) pair."""
            if len(key) == 2 and isinstance(key[1], int) and key[0] in (
                    "pe", "dve", "act", "gps", "hid", "hbc", "bias",
                    "l0", "l1", "l2", "wt", "outd"):
                sname, val = key
            else:
                if self.emit and key not in ev:
                    raise KeyError(f"wait on unknown event {key}")
                sname, val = ev.get(key, (None, 0))
            if val <= 0 or sname is None:
                return
            if self.hwm.get(sname, -1) >= val:
                return
            self.hwm[sname] = val
            if self.emit:
                self.eng.wait_ge(sems[sname], val)

        def op(self, fn, inc=1, sem=None, drain=None):
            sname = sem or self.name
            counts[sname] = counts.get(sname, 0) + inc
            if self.emit:
                do_drain = self.auto_drain if drain is None else drain
                if do_drain and not self.first_op:
                    self.eng.drain()
                inst = fn()
                inst.then_inc(sems[sname], inc)
            self.first_op = False

        def mark(self, *key, sem=None):
            sname = sem or self.name
            ev[(self.name,) + tuple(key)] = (sname, counts.get(sname, 0))

    DMA, PE, DVE, ACT, GPS = Prog("dma"), Prog("pe"), Prog("dve"), Prog("act"), Prog("gps")

    bias_src = b_d[:]
    bias_bcast = bass.AP(
        tensor=bias_src.tensor,
        offset=bias_src.offset,
        ap=[[0, BPC]] + list(bias_src.ap),
    )
    def hidR_bcast(b):
        src = hid16_d[b]
        return bass.AP(tensor=src.tensor, offset=src.offset,
                       ap=[[0, 128]] + list(src.ap))

    def prog_gps():
        g = GPS.eng if GPS.emit else None
        GPS.op(lambda: g.memset(ones_col.ap(), 1.0))
        GPS.op(lambda: g.memset(ones128.ap(), 1.0))
        GPS.op(lambda: g.memset(ident.ap(), 0.0))
        GPS.op(lambda: g.affine_select(
            out=ident.ap(), in_=ident.ap(),
            compare_op=mybir.AluOpType.not_equal, fill=1.0, base=0,
            pattern=[[-1, 128]], channel_multiplier=1))
        GPS.mark("setup")
        # one-level fold of the short quad: prod[:, :, 0:512] + [:, :, 512:]
        fq = GPS_FOLD_QS[0]
        for b in range(BPC):
            GPS.wait(("dve", "multq", b, fq))
            if b >= 2:
                # parity fold buffer: reused two batches back
                GPS.wait(("act", "acc", b - 2, 4 * fq + 3))
            for j in range(TQ):
                GPS.op(lambda b=b, j=j: g.tensor_add(
                    fold1[b % 2].ap()[:, j, :],
                    prodQ[fq].ap()[:, j, 0:512],
                    prodQ[fq].ap()[:, j, 512:1024]), drain=False)
            # drained dummy: its inc proves the adds above completed
            GPS.op(lambda: g.memset(gdum.ap(), 0.0), drain=True)
            GPS.mark("fold", b, fq)

    def prog_dma():
        d = DMA.eng if DMA.emit else None
        for b in range(BPC):
            if b >= NL:
                DMA.wait(("pe", "e2", b - NL))
            if b >= 2:
                DMA.wait(("dve", "multq", b - 2, NQ - 1))  # hidR slot reuse
            DMA.op(lambda b=b: d.dma_start(
                out=hidR.ap()[:, b % 2, :], in_=hidR_bcast(b)),
                inc=16, sem="hbc")
            DMA.mark("hR", b, sem="hbc")
            src = lstm_d[b]
            for q in range(NQ):
                DMA.op(lambda src=src, b=b, q=q: d.dma_start(
                    out=L[b % NL].ap()[:, TQ * q:TQ * (q + 1), :],
                    in_=src[:, TQ * q:TQ * (q + 1), :]),
                    inc=16, sem=f"l{b % NL}")
                DMA.mark("Lq", b, q, sem=f"l{b % NL}")
            DMA.mark("L", b, sem=f"l{b % NL}")
            if b == 0:
                DMA.op(lambda: d.dma_start(out=hid, in_=hid_d[:]),
                       inc=16, sem="hid")
                DMA.mark("hid", sem="hid")
            if b == 2:
                DMA.op(lambda: d.dma_start(out=WT.ap(), in_=wt_d[:]),
                       inc=16, sem="wt")
                DMA.mark("wt", sem="wt")
                DMA.op(lambda: d.dma_start(out=bias, in_=bias_bcast),
                       inc=16, sem="bias")
                DMA.mark("bias", sem="bias")
        DMA.wait(("dve", "bias_hi"))
        DMA.op(lambda: d.dma_start(out=out_d[:], in_=out_sb), inc=16, sem="outd")
        DMA.wait(("outd", counts.get("outd", 0)))

    def prog_pe():
        p = PE.eng if PE.emit else None
        PE.wait(("gps", "setup"))
        PE.wait(("dma", "hid"))
        # hidden^T -> CT chunks 0..7 staging (psum)
        for c in range(HCH):
            PE.op(lambda c=c: p.transpose(
                ctcols8[:, c, :], hid[0:BPC, c * 128:(c + 1) * 128],
                ident.ap()[0:BPC, 0:BPC]))
        PE.mark("hidT")
        PE.wait(("dve", "rmax", 0))
        PE.op(lambda: p.transpose(mpT, mp.ap(), ident.ap()))
        PE.mark("transp", 0)
        for b in range(BPC):
            PE.wait(("dve", "rmax2", b))
            PE.op(lambda b=b: p.matmul(
                negM_bc, lhsT=ones_col.ap(), rhs=negM1[b % 2],
                start=True, stop=True))
            PE.mark("bcast", b)
            if b >= 1:
                PE.wait(("dve", "recip", b - 1))   # Zps slot reuse
            PE.wait(("act", "exp", b))
            PE.op(lambda b=b: p.matmul(
                Zps, lhsT=zp[b % 2].ap(), rhs=ones128.ap(),
                start=True, stop=True))
            PE.mark("z", b)
            # einsum2: attn_unnorm = sum_s w[s] * L[s, :]
            if b >= 1:
                PE.wait(("act", "cphi", b - 1))    # acc bank reuse
            for t in range(T):
                PE.op(lambda b=b, t=t: p.matmul(
                    acc_lo.ap()[0:1, :],
                    lhsT=wexp[b % 2].ap()[:, t:t + 1],
                    rhs=L[b % NL].ap()[:, t, 0:512],
                    start=(t == 0), stop=(t == T - 1)))
                PE.op(lambda b=b, t=t: p.matmul(
                    acc_hi.ap()[0:1, :],
                    lhsT=wexp[b % 2].ap()[:, t:t + 1],
                    rhs=L[b % NL].ap()[:, t, 512:1024],
                    start=(t == 0), stop=(t == T - 1)))
            PE.mark("e2", b)
            if b + 1 < BPC:
                PE.wait(("dve", "rmax", b + 1))
                PE.op(lambda: p.transpose(mpT, mp.ap(), ident.ap()))
                PE.mark("transp", b + 1)
            # attn row -> columns (chunk transposes via K=1 matmuls)
            PE.wait(("act", "cphi", b))
            if b >= 1:
                PE.wait(("act", "ctcp", b - 1))
            for c in range(HCH):
                PE.op(lambda b=b, c=c: p.transpose(
                    ctcols[:, c:c + 1],
                    attn_sb[b % 2][0:1, c * 128:(c + 1) * 128],
                    ones_col.ap()[0:1, 0:1]))
            PE.mark("attnT", b)
        # final projection (fp16): combined^T @ W^T chunks
        PE.wait(("act", "ctcp", BPC - 1))
        PE.wait(("dma", "wt"))
        for c in range(NCH):
            PE.op(lambda c=c: p.matmul(
                acc_lo.ap()[0:BPC, :],
                lhsT=CT.ap()[:, c, :],
                rhs=WT.ap()[:, c, 0:512],
                start=(c == 0), stop=(c == NCH - 1)))
            PE.op(lambda c=c: p.matmul(
                acc_hi.ap()[0:BPC, :],
                lhsT=CT.ap()[:, c, :],
                rhs=WT.ap()[:, c, 512:1024],
                start=(c == 0), stop=(c == NCH - 1)))
        PE.mark("final")

    def prog_dve():
        v = DVE.eng if DVE.emit else None
        # CT hidden columns: psum staging -> CT (f32 -> fp16 convert)
        DVE.wait(("pe", "hidT"))
        DVE.op(lambda: v.tensor_copy(CT.ap()[:, 0:HCH, :], ctcols8))
        DVE.mark("cth")
        def emit_multq(b, q):
            DVE.wait(("dma", "hR", b))
            DVE.wait(("dma", "Lq", b, q))
            if b >= 1:
                # prodQ[q] reuse: wait for its last reader, prev batch
                if q in GPS_FOLD_QS:
                    DVE.wait(("gps", "fold", b - 1, q))
                elif 4 * q < NACT:
                    DVE.wait(("act", "acc", b - 1, 4 * q + 3))
            DVE.op(lambda b=b, q=q: v.tensor_mul(
                prodQ[q].ap(),
                L[b % NL].ap()[:, TQ * q:TQ * (q + 1), :],
                hidR.ap()[:, b % 2:b % 2 + 1, :].broadcast_to((128, TQ, H))),
                drain=False)
            DVE.mark("multq", b, q)

        def emit_qred(b):
            if b >= 2:
                DVE.wait(("act", "exp", b - 2))   # scores slot reuse
            # DVE-owned score columns NACT..15 (one quad)
            DVE.op(lambda b=b: v.tensor_reduce(
                scores[b % 2].ap()[:, NACT:T],
                prodQ[NACT // 4].ap(),
                axis=mybir.AxisListType.X, op=mybir.AluOpType.add))

        # software-pipelined: next batch's first mults run while this
        # batch's rmax waits on ACT's accum tail
        for q in range(NQ):
            emit_multq(0, q)
        emit_qred(0)
        for b in range(BPC):
            if b + 1 < BPC:
                emit_multq(b + 1, 0)
            if b >= 1:
                DVE.wait(("pe", "transp", b - 1))   # mp slot reuse
            DVE.wait(("act", "acc", b, ACT_ORDER[-1]))
            DVE.op(lambda b=b: v.reduce_max(
                mp.ap(), scores[b % 2].ap(), axis=mybir.AxisListType.X))
            DVE.mark("rmax", b)
            if b + 1 < BPC:
                emit_multq(b + 1, 1)
            DVE.wait(("pe", "transp", b))
            DVE.op(lambda b=b: v.reduce_max(
                negM1[b % 2], mpT, axis=mybir.AxisListType.X, negate=True))
            DVE.mark("rmax2", b)
            if b + 1 < BPC:
                emit_multq(b + 1, 2)
                DVE.wait(("pe", "z", b))
                DVE.op(lambda b=b: v.reciprocal(rZ[b % 2], Zps),
                       drain=False)
                DVE.mark("recip", b)
                emit_multq(b + 1, 3)
                emit_qred(b + 1)
        DVE.wait(("pe", "z", BPC - 1))
        DVE.op(lambda: v.reciprocal(rZ[(BPC - 1) % 2], Zps))
        DVE.mark("recip", BPC - 1)
        # final bias adds
        DVE.wait(("pe", "final"))
        DVE.op(lambda: v.tensor_add(
            out_sb[:, 0:512], acc_lo.ap()[0:BPC, :], bias[:, 0:512]))
        DVE.mark("bias_lo")
        DVE.wait(("dma", "bias"))
        DVE.op(lambda: v.tensor_add(
            out_sb[:, 512:1024], acc_hi.ap()[0:BPC, :], bias[:, 512:1024]))
        DVE.mark("bias_hi")

    def prog_act():
        a = ACT.eng if ACT.emit else None
        Copy = mybir.ActivationFunctionType.Copy
        Exp = mybir.ActivationFunctionType.Exp
        def emit_exp(eb):
            # negM copy + exp for batch eb (deferred into the next block)
            ACT.wait(("pe", "bcast", eb))
            ACT.op(lambda eb=eb: a.activation(
                out=negM[eb % 2].ap(), in_=negM_bc, func=Copy))
            ACT.mark("negMcp", eb)
            if eb >= 2:
                ACT.wait(("pe", "e2", eb - 2))    # wexp/zp slot reuse
            ACT.op(lambda eb=eb: a.activation(
                out=wexp[eb % 2].ap(), in_=scores[eb % 2].ap(), func=Exp,
                bias=negM[eb % 2].ap(), scale=1.0, accum_out=zp[eb % 2].ap()))
            ACT.mark("exp", eb)

        for b in range(BPC):
            for i, t in enumerate(ACT_ORDER):
                if t // 4 in GPS_FOLD_QS:
                    # GPS pre-folded to 512 wide
                    ACT.wait(("gps", "fold", b, t // 4))
                    ACT.op(lambda b=b, t=t: a.activation(
                        out=dmy.ap()[:, t:t + 1].broadcast_to((128, 512)),
                        in_=fold1[b % 2].ap()[:, t % 4, :],
                        func=Copy, accum_out=scores[b % 2].ap()[:, t:t + 1]),
                        drain=False)
                else:
                    ACT.wait(("dve", "multq", b, t // 4))
                    ACT.op(lambda b=b, t=t: a.activation(
                        out=dmy.ap()[:, t:t + 1].broadcast_to((128, H)),
                        in_=prodQ[t // 4].ap()[:, t % 4, :],
                        func=Copy, accum_out=scores[b % 2].ap()[:, t:t + 1]),
                        drain=(i == 0))
                ACT.mark("acc", b, t)
                if i == 2 and b >= 1:
                    emit_exp(b - 1)
                if i == 4 and b >= 2:
                    ACT.wait(("pe", "attnT", b - 2))
                    ACT.op(lambda b=b: a.activation(
                        out=CT.ap()[:, HCH:NCH, b - 2], in_=ctcols,
                        func=Copy))
                    ACT.mark("ctcp", b - 2)
            if b >= 1:
                # after all accums: e2(b-1) is long done, and these hide the
                # rmax->bcast chain latency before exp(b)
                ACT.wait(("pe", "e2", b - 1))
                ACT.wait(("dve", "recip", b - 1))
                ACT.op(lambda b=b: a.activation(
                    out=attn_sb[(b - 1) % 2][0:1, 0:512],
                    in_=acc_lo.ap()[0:1, :],
                    func=Copy, scale=rZ[(b - 1) % 2]))
                ACT.mark("cplo", b - 1)
                ACT.op(lambda b=b: a.activation(
                    out=attn_sb[(b - 1) % 2][0:1, 512:1024],
                    in_=acc_hi.ap()[0:1, :],
                    func=Copy, scale=rZ[(b - 1) % 2]))
                ACT.mark("cphi", b - 1)
        emit_exp(BPC - 1)
        for b in (BPC - 1,):
            ACT.wait(("pe", "e2", b))
            ACT.wait(("dve", "recip", b))
            ACT.op(lambda b=b: a.activation(
                out=attn_sb[b % 2][0:1, 0:512], in_=acc_lo.ap()[0:1, :],
                func=Copy, scale=rZ[b % 2]))
            ACT.mark("cplo", b)
            ACT.op(lambda b=b: a.activation(
                out=attn_sb[b % 2][0:1, 512:1024], in_=acc_hi.ap()[0:1, :],
                func=Copy, scale=rZ[b % 2]))
            ACT.mark("cphi", b)
        for b in (BPC - 2, BPC - 1):
            ACT.wait(("pe", "attnT", b))
            ACT.op(lambda b=b: a.activation(
                out=CT.ap()[:, HCH:NCH, b], in_=ctcols, func=Copy))
            ACT.mark("ctcp", b)

    progs = [
        (GPS, prog_gps), (DMA, prog_dma), (PE, prog_pe),
        (DVE, prog_dve), (ACT, prog_act),
    ]

    # pass 1: count
    for pr, fn in progs:
        pr.begin(emit=False)
        fn()

    # pass 2: emit
    counts.clear()
    sem_names = ["pe", "dve", "act", "gps", "hid", "hbc", "bias",
                 "l0", "l1", "l2", "wt", "outd"]
    with nc.Block() as block:
        for sn in sem_names:
            sems[sn] = nc.alloc_semaphore(name=f"{sn}_sem")

        @block.gpsimd
        def _(eng):
            GPS.begin(eng=eng, emit=True)
            prog_gps()

        @block.sync
        def _(eng):
            DMA.begin(eng=eng, emit=True)
            prog_dma()

        @block.tensor
        def _(eng):
            PE.begin(eng=eng, emit=True)
            prog_pe()

        @block.vector
        def _(eng):
            DVE.begin(eng=eng, emit=True)
            prog_dve()

        @block.scalar
        def _(eng):
            ACT.begin(eng=eng, emit=True)
            prog_act()

    return nc


def kernel(lstm_output, hidden, W_combine, b_combine):
    global _cached_nc, last_results
    lstm_output = np.asarray(lstm_output, dtype=np.float32)
    hidden = np.asarray(hidden, dtype=np.float32)
    W_combine = np.asarray(W_combine, dtype=np.float32)
    b_combine = np.asarray(b_combine, dtype=np.float32)

    if _cached_nc is None:
        _cached_nc = _build_program()
    nc = _cached_nc

    # fp16 partition-major relayout: [B, S, H] -> [B, 128, T, H]
    l16 = lstm_output.astype(np.float16).reshape(B, T, 128, H).transpose(0, 2, 1, 3)
    wt16 = np.ascontiguousarray(
        W_combine.T.astype(np.float16).reshape(NCH, 128, H).transpose(1, 0, 2))
    hid16 = hidden.astype(np.float16)

    in_maps = []
    for i in range(NCORES):
        sl = slice(i * BPC, (i + 1) * BPC)
        in_maps.append({
            "lstm16": np.ascontiguousarray(l16[sl]),
            "hidden": np.ascontiguousarray(hidden[sl]),
            "hidden16": np.ascontiguousarray(hid16[sl]),
            "w_t16": wt16,
            "b_combine": b_combine,
        })
    res = run_bass_kernel_spmd(nc, in_maps, core_ids=list(range(NCORES)))
    last_results = res
    return np.concatenate([res.results[i]["out"] for i in range(NCORES)], axis=0)


# revision 41
# speedup vs baseline: 1.1883x; 1.1883x over previous
"""Trainium2 Bass kernel for nn_Attention (dense_transformer, ridge regime).

Computation per batch b:
    scores[s]  = <lstm_output[b,s,:], hidden[b,:]>          # [S]
    w          = softmax(scores)                            # [S]
    attn[h]    = sum_s w[s] * lstm_output[b,s,h]            # [H]
    out[b]     = [hidden[b], attn] @ W_combine.T + b_combine

Sharding: data-parallel over batch B=64 across 8 cores (8 batches/core).

v3 design (fp16 data path; measured rel-err ~1.2e-3 vs 2e-2 gate):
  - Host converts lstm_output to fp16, partition-major [BPC, 128, T, H]:
    each batch one contiguous-per-partition 4MB DMA in 1MB quarters,
    L triple-buffered. HBM/core: 32MB lstm + 4MB W.T + 2MB hidR + small.
  - hidR (hidden replicated across partitions) comes from one broadcast
    DMA (stride-0 partition read of hidden16) -- no sel-matmul staging.
  - Scores per batch: 4 quad-multiplies on DVE (fp16 2x, one per DMA
    quarter, in1 = hidR stride-0 broadcast over the t dim), each into its
    own prodQ buffer so ACT never paces DVE. Columns 0-10 reduced by ACT
    accum-copies; 11-15 by DVE (single + quad tensor_reduce).
  - Softmax max chain via PE transposes; exp on ACT (fp16 wexp out, f32 Z
    accum); einsum2 and final projection as fp16 PE matmuls; transposes
    stay f32 into PSUM, converting on the ACT/DVE copy-out.
  - W.T fp16 [128, 16, 1024] loads mid-stream into its own SBUF buffer.
"""

import numpy as np

import concourse.bass as bass
from concourse import bass_isa, library_config, mybir
from concourse.bass_utils import run_bass_kernel_spmd

F32 = mybir.dt.float32
F16 = mybir.dt.float16

B, S, H = 64, 2048, 1024
NCORES = 8
BPC = B // NCORES          # batches per core
T = S // 128               # s-tiles per batch
NCH = (2 * H) // 128       # 16 chunks of the combined dim
HCH = H // 128             # 8 chunks of one H
NQ = 4                     # DMA quarters / mult quads per batch
TQ = T // NQ               # t-tiles per quarter
NL = 3                     # L buffer slots (triple buffer)

NACT = 12                  # score cols 0..NACT-1 on ACT; 12-15 on DVE
GPS_FOLD_QS = (1,)         # quads whose cols ACT reads GPS-pre-folded to 512
ACT_ORDER = [0, 1, 2, 3, 8, 9, 10, 11, 4, 5, 6, 7]  # fulls first, then shorts

_cached_nc = None
last_results = None


def _build_program():
    nc = bass.Bass()

    lstm_d = nc.declare_dram_parameter("lstm16", [BPC, 128, T, H], F16, isOutput=False)
    hid_d = nc.declare_dram_parameter("hidden", [BPC, H], F32, isOutput=False)
    hid16_d = nc.declare_dram_parameter("hidden16", [BPC, H], F16, isOutput=False)
    wt_d = nc.declare_dram_parameter("w_t16", [128, NCH, H], F16, isOutput=False)
    b_d = nc.declare_dram_parameter("b_combine", [H], F32, isOutput=False)
    out_d = nc.declare_dram_parameter("out", [BPC, H], F32, isOutput=True)

    # ---- SBUF ----
    L = [nc.alloc_sbuf_tensor(f"L{i}", [128, T, H], F16) for i in range(NL)]  # 3x4MB
    WT = nc.alloc_sbuf_tensor("WT", [128, NCH, H], F16)                       # 4MB
    hid_t = nc.alloc_sbuf_tensor("hid", [BPC, H], F32)
    hid = hid_t.ap()
    bias_t = nc.alloc_sbuf_tensor("bias", [BPC, H], F32)
    bias = bias_t.ap()
    out_t = nc.alloc_sbuf_tensor("out_sb", [BPC, H], F32)
    out_sb = out_t.ap()
    hidR = nc.alloc_sbuf_tensor("hidR", [128, 2, H], F16)  # rotating bcast hidden
    prodQ = [nc.alloc_sbuf_tensor(f"prodQ{i}", [128, TQ, H], F16) for i in range(NQ)]
    fold1 = [nc.alloc_sbuf_tensor(f"fold1_{i}", [128, TQ, 512], F32)
             for i in range(2)]   # parity-doubled fold buffer
    gdum = nc.alloc_sbuf_tensor("gdum", [1, 1], F32)
    dmy = nc.alloc_sbuf_tensor("dmy", [128, T], F32)
    CT = nc.alloc_sbuf_tensor("CT", [128, NCH, BPC], F16)     # combined^T
    scores = [nc.alloc_sbuf_tensor(f"scores{i}", [128, T], F32) for i in range(2)]
    wexp = [nc.alloc_sbuf_tensor(f"wexp{i}", [128, T], F16) for i in range(2)]
    zp = [nc.alloc_sbuf_tensor(f"zp{i}", [128, 1], F32) for i in range(2)]
    mp = nc.alloc_sbuf_tensor("mp", [128, 1], F32)
    negM1_t = nc.alloc_sbuf_tensor("negM1s", [1, 2], F32)
    negM1 = [negM1_t.ap()[0:1, i:i + 1] for i in range(2)]
    negM = [nc.alloc_sbuf_tensor(f"negM{i}", [128, 1], F32) for i in range(2)]
    rZ_t = nc.alloc_sbuf_tensor("rZs", [1, 2], F32)
    rZ = [rZ_t.ap()[0:1, i:i + 1] for i in range(2)]
    ones128 = nc.alloc_sbuf_tensor("ones128", [128, 1], F32)
    attn2 = nc.alloc_sbuf_tensor("attn2", [1, 2 * H], F32)
    attn_sb = [attn2.ap()[0:1, i * H:(i + 1) * H] for i in range(2)]
    ones_col = nc.alloc_sbuf_tensor("ones_col", [1, 128], F32)
    ident = nc.alloc_sbuf_tensor("ident", [128, 128], F32)

    # ---- PSUM ----
    acc_lo = nc.alloc_psum_tensor("acc_lo", [BPC, 512], F32)  # einsum2 row 0 / final
    acc_hi = nc.alloc_psum_tensor("acc_hi", [BPC, 512], F32)
    ct8_t = nc.alloc_psum_tensor("ct8", [128, HCH, BPC], F32) # hidT staging
    ctc_t = nc.alloc_psum_tensor("ctc", [128, 512], F32)      # attnT transposes
    mpT_t = nc.alloc_psum_tensor("mpT", [1, 128], F32)        # transp target
    negM_t = nc.alloc_psum_tensor("negMbc", [128, 1], F32)    # bcast mm target
    Zps_t = nc.alloc_psum_tensor("Zps", [1, 1], F32)          # Z mm target
    mpT = mpT_t.ap()
    negM_bc = negM_t.ap()
    Zps = Zps_t.ap()
    ctcols8 = ct8_t.ap()
    ctcols = ctc_t.ap()[:, 0:HCH]

    # ---------------- two-pass emission ----------------
    ev = {}
    sems = {}
    counts = {}

    class Prog:
        def __init__(self, name):
            self.name = name
            self.emit = False
            self.eng = None
            self.hwm = {}
            self.auto_drain = name in ("dve", "act", "gps")
            self.first_op = True

        def begin(self, eng=None, emit=False):
            self.emit = emit
            self.eng = eng
            self.hwm = {}
            self.first_op = True

        def wait(self, key):
            """key: event tuple, or (sem_name, value) pair."""
            if len(key) == 2 and isinstance(key[1], int) and key[0] in (
                    "pe", "dve", "act", "gps", "hid", "hbc", "bias",
                    "l0", "l1", "l2", "wt", "outd"):
                sname, val = key
            else:
                if self.emit and key not in ev:
                    raise KeyError(f"wait on unknown event {key}")
                sname, val = ev.get(key, (None, 0))
            if val <= 0 or sname is None:
                return
            if self.hwm.get(sname, -1) >= val:
                return
            self.hwm[sname] = val
            if self.emit:
                self.eng.wait_ge(sems[sname], val)

        def op(self, fn, inc=1, sem=None, drain=None):
            sname = sem or self.name
            counts[sname] = counts.get(sname, 0) + inc
            if self.emit:
                do_drain = self.auto_drain if drain is None else drain
                if do_drain and not self.first_op:
                    self.eng.drain()
                inst = fn()
                inst.then_inc(sems[sname], inc)
            self.first_op = False

        def mark(self, *key, sem=None):
            sname = sem or self.name
            ev[(self.name,) + tuple(key)] = (sname, counts.get(sname, 0))

    DMA, PE, DVE, ACT, GPS = Prog("dma"), Prog("pe"), Prog("dve"), Prog("act"), Prog("gps")

    bias_src = b_d[:]
    bias_bcast = bass.AP(
        tensor=bias_src.tensor,
        offset=bias_src.offset,
        ap=[[0, BPC]] + list(bias_src.ap),
    )
    def hidR_bcast(b):
        src = hid16_d[b]
        return bass.AP(tensor=src.tensor, offset=src.offset,
                       ap=[[0, 128]] + list(src.ap))

    def prog_gps():
        g = GPS.eng if GPS.emit else None
        GPS.op(lambda: g.memset(ones_col.ap(), 1.0))
        GPS.op(lambda: g.memset(ones128.ap(), 1.0))
        GPS.op(lambda: g.memset(ident.ap(), 0.0))
        GPS.op(lambda: g.affine_select(
            out=ident.ap(), in_=ident.ap(),
            compare_op=mybir.AluOpType.not_equal, fill=1.0, base=0,
            pattern=[[-1, 128]], channel_multiplier=1))
        GPS.mark("setup")
        # one-level fold of the short quad: prod[:, :, 0:512] + [:, :, 512:]
        fq = GPS_FOLD_QS[0]
        for b in range(BPC):
            GPS.wait(("dve", "multq", b, fq))
            if b >= 2:
                # parity fold buffer: reused two batches back
                GPS.wait(("act", "acc", b - 2, 4 * fq + 3))
            for j in range(TQ):
                GPS.op(lambda b=b, j=j: g.tensor_add(
                    fold1[b % 2].ap()[:, j, :],
                    prodQ[fq].ap()[:, j, 0:512],
                    prodQ[fq].ap()[:, j, 512:1024]), drain=False)
            # drained dummy: its inc proves the adds above completed
            GPS.op(lambda: g.memset(gdum.ap(), 0.0), drain=True)
            GPS.mark("fold", b, fq)

    def prog_dma():
        d = DMA.eng if DMA.emit else None
        for b in range(BPC):
            if b >= NL:
                DMA.wait(("pe", "e2", b - NL))
            if b >= 2:
                DMA.wait(("dve", "multq", b - 2, NQ - 1))  # hidR slot reuse
            DMA.op(lambda b=b: d.dma_start(
                out=hidR.ap()[:, b % 2, :], in_=hidR_bcast(b)),
                inc=16, sem="hbc")
            DMA.mark("hR", b, sem="hbc")
            src = lstm_d[b]
            for q in range(NQ):
                DMA.op(lambda src=src, b=b, q=q: d.dma_start(
                    out=L[b % NL].ap()[:, TQ * q:TQ * (q + 1), :],
                    in_=src[:, TQ * q:TQ * (q + 1), :]),
                    inc=16, sem=f"l{b % NL}")
                DMA.mark("Lq", b, q, sem=f"l{b % NL}")
            DMA.mark("L", b, sem=f"l{b % NL}")
            if b == 0:
                DMA.op(lambda: d.dma_start(out=hid, in_=hid_d[:]),
                       inc=16, sem="hid")
                DMA.mark("hid", sem="hid")
            if b == 2:
                DMA.op(lambda: d.dma_start(out=WT.ap(), in_=wt_d[:]),
                       inc=16, sem="wt")
                DMA.mark("wt", sem="wt")
                DMA.op(lambda: d.dma_start(out=bias, in_=bias_bcast),
                       inc=16, sem="bias")
                DMA.mark("bias", sem="bias")
        DMA.wait(("dve", "bias_hi"))
        DMA.op(lambda: d.dma_start(out=out_d[:], in_=out_sb), inc=16, sem="outd")
        DMA.wait(("outd", counts.get("outd", 0)))

    def prog_pe():
        p = PE.eng if PE.emit else None
        PE.wait(("gps", "setup"))
        PE.wait(("dma", "hid"))
        # hidden^T -> CT chunks 0..7 staging (psum)
        for c in range(HCH):
            PE.op(lambda c=c: p.transpose(
                ctcols8[:, c, :], hid[0:BPC, c * 128:(c + 1) * 128],
                ident.ap()[0:BPC, 0:BPC]))
        PE.mark("hidT")
        PE.wait(("dve", "rmax", 0))
        PE.op(lambda: p.transpose(mpT, mp.ap(), ident.ap()))
        PE.mark("transp", 0)
        for b in range(BPC):
            PE.wait(("dve", "rmax2", b))
            PE.op(lambda b=b: p.matmul(
                negM_bc, lhsT=ones_col.ap(), rhs=negM1[b % 2],
                start=True, stop=True))
            PE.mark("bcast", b)
            if b >= 1:
                PE.wait(("dve", "recip", b - 1))   # Zps slot reuse
            PE.wait(("act", "exp", b))
            PE.op(lambda b=b: p.matmul(
                Zps, lhsT=zp[b % 2].ap(), rhs=ones128.ap(),
                start=True, stop=True))
            PE.mark("z", b)
            # einsum2: attn_unnorm = sum_s w[s] * L[s, :]
            if b >= 1:
                PE.wait(("act", "cphi", b - 1))    # acc bank reuse
            for t in range(T):
                PE.op(lambda b=b, t=t: p.matmul(
                    acc_lo.ap()[0:1, :],
                    lhsT=wexp[b % 2].ap()[:, t:t + 1],
                    rhs=L[b % NL].ap()[:, t, 0:512],
                    start=(t == 0), stop=(t == T - 1)))
                PE.op(lambda b=b, t=t: p.matmul(
                    acc_hi.ap()[0:1, :],
                    lhsT=wexp[b % 2].ap()[:, t:t + 1],
                    rhs=L[b % NL].ap()[:, t, 512:1024],
                    start=(t == 0), stop=(t == T - 1)))
            PE.mark("e2", b)
            if b + 1 < BPC:
                PE.wait(("dve", "rmax", b + 1))
                PE.op(lambda: p.transpose(mpT, mp.ap(), ident.ap()))
                PE.mark("transp", b + 1)
            # attn row -> columns (chunk transposes via K=1 matmuls)
            PE.wait(("act", "cphi", b))
            if b >= 1:
                PE.wait(("act", "ctcp", b - 1))
            for c in range(HCH):
                PE.op(lambda b=b, c=c: p.transpose(
                    ctcols[:, c:c + 1],
                    attn_sb[b % 2][0:1, c * 128:(c + 1) * 128],
                    ones_col.ap()[0:1, 0:1]))
            PE.mark("attnT", b)
        # final projection (fp16): combined^T @ W^T chunks
        PE.wait(("act", "ctcp", BPC - 1))
        PE.wait(("dma", "wt"))
        for c in range(NCH):
            PE.op(lambda c=c: p.matmul(
                acc_lo.ap()[0:BPC, :],
                lhsT=CT.ap()[:, c, :],
                rhs=WT.ap()[:, c, 0:512],
                start=(c == 0), stop=(c == NCH - 1)))
            PE.op(lambda c=c: p.matmul(
                acc_hi.ap()[0:BPC, :],
                lhsT=CT.ap()[:, c, :],
                rhs=WT.ap()[:, c, 512:1024],
                start=(c == 0), stop=(c == NCH - 1)))
        PE.mark("final")

    def prog_dve():
        v = DVE.eng if DVE.emit else None
        # CT hidden columns: psum staging -> CT (f32 -> fp16 convert)
        DVE.wait(("pe", "hidT"))
        DVE.op(lambda: v.tensor_copy(CT.ap()[:, 0:HCH, :], ctcols8))
        DVE.mark("cth")
        def emit_multq(b, q):
            DVE.wait(("dma", "hR", b))
            DVE.wait(("dma", "Lq", b, q))
            if b >= 1:
                # prodQ[q] reuse: wait for its last reader, prev batch
                if q in GPS_FOLD_QS:
                    DVE.wait(("gps", "fold", b - 1, q))
                elif 4 * q < NACT:
                    DVE.wait(("act", "acc", b - 1, 4 * q + 3))
            DVE.op(lambda b=b, q=q: v.tensor_mul(
                prodQ[q].ap(),
                L[b % NL].ap()[:, TQ * q:TQ * (q + 1), :],
                hidR.ap()[:, b % 2:b % 2 + 1, :].broadcast_to((128, TQ, H))),
                drain=False)
            DVE.mark("multq", b, q)

        def emit_qred(b):
            if b >= 2:
                DVE.wait(("act", "exp", b - 2))   # scores slot reuse
            # DVE-owned score columns NACT..15 (one quad)
            DVE.op(lambda b=b: v.tensor_reduce(
                scores[b % 2].ap()[:, NACT:T],
                prodQ[NACT // 4].ap(),
                axis=mybir.AxisListType.X, op=mybir.AluOpType.add))

        # software-pipelined: next batch's first mults run while this
        # batch's rmax waits on ACT's accum tail
        for q in range(NQ):
            emit_multq(0, q)
        emit_qred(0)
        for b in range(BPC):
            if b + 1 < BPC:
                emit_multq(b + 1, 0)
            if b >= 1:
                DVE.wait(("pe", "transp", b - 1))   # mp slot reuse
            DVE.wait(("act", "acc", b, ACT_ORDER[-1]))
            DVE.op(lambda b=b: v.reduce_max(
                mp.ap(), scores[b % 2].ap(), axis=mybir.AxisListType.X))
            DVE.mark("rmax", b)
            DVE.wait(("pe", "transp", b))
            DVE.op(lambda b=b: v.reduce_max(
                negM1[b % 2], mpT, axis=mybir.AxisListType.X, negate=True))
            DVE.mark("rmax2", b)
            if b + 1 < BPC:
                emit_multq(b + 1, 1)
                emit_multq(b + 1, 2)
                DVE.wait(("pe", "z", b))
                DVE.op(lambda b=b: v.reciprocal(rZ[b % 2], Zps),
                       drain=False)
                DVE.mark("recip", b)
                emit_multq(b + 1, 3)
                emit_qred(b + 1)
        DVE.wait(("pe", "z", BPC - 1))
        DVE.op(lambda: v.reciprocal(rZ[(BPC - 1) % 2], Zps))
        DVE.mark("recip", BPC - 1)
        # final bias adds
        DVE.wait(("pe", "final"))
        DVE.op(lambda: v.tensor_add(
            out_sb[:, 0:512], acc_lo.ap()[0:BPC, :], bias[:, 0:512]))
        DVE.mark("bias_lo")
        DVE.wait(("dma", "bias"))
        DVE.op(lambda: v.tensor_add(
            out_sb[:, 512:1024], acc_hi.ap()[0:BPC, :], bias[:, 512:1024]))
        DVE.mark("bias_hi")

    def prog_act():
        a = ACT.eng if ACT.emit else None
        Copy = mybir.ActivationFunctionType.Copy
        Exp = mybir.ActivationFunctionType.Exp
        def emit_exp(eb):
            # negM copy + exp for batch eb (deferred into the next block)
            ACT.wait(("pe", "bcast", eb))
            ACT.op(lambda eb=eb: a.activation(
                out=negM[eb % 2].ap(), in_=negM_bc, func=Copy))
            ACT.mark("negMcp", eb)
            if eb >= 2:
                ACT.wait(("pe", "e2", eb - 2))    # wexp/zp slot reuse
            ACT.op(lambda eb=eb: a.activation(
                out=wexp[eb % 2].ap(), in_=scores[eb % 2].ap(), func=Exp,
                bias=negM[eb % 2].ap(), scale=1.0, accum_out=zp[eb % 2].ap()))
            ACT.mark("exp", eb)

        for b in range(BPC):
            for i, t in enumerate(ACT_ORDER):
                if t // 4 in GPS_FOLD_QS:
                    # GPS pre-folded to 512 wide
                    ACT.wait(("gps", "fold", b, t // 4))
                    ACT.op(lambda b=b, t=t: a.activation(
                        out=dmy.ap()[:, t:t + 1].broadcast_to((128, 512)),
                        in_=fold1[b % 2].ap()[:, t % 4, :],
                        func=Copy, accum_out=scores[b % 2].ap()[:, t:t + 1]),
                        drain=False)
                else:
                    ACT.wait(("dve", "multq", b, t // 4))
                    ACT.op(lambda b=b, t=t: a.activation(
                        out=dmy.ap()[:, t:t + 1].broadcast_to((128, H)),
                        in_=prodQ[t // 4].ap()[:, t % 4, :],
                        func=Copy, accum_out=scores[b % 2].ap()[:, t:t + 1]),
                        drain=(i == 0))
                ACT.mark("acc", b, t)
                if i == 2 and b >= 1:
                    emit_exp(b - 1)
                if i == 4 and b >= 2:
                    ACT.wait(("pe", "attnT", b - 2))
                    ACT.op(lambda b=b: a.activation(
                        out=CT.ap()[:, HCH:NCH, b - 2], in_=ctcols,
                        func=Copy))
                    ACT.mark("ctcp", b - 2)
            if b >= 1:
                # after all accums: e2(b-1) is long done, and these hide the
                # rmax->bcast chain latency before exp(b)
                ACT.wait(("pe", "e2", b - 1))
                ACT.wait(("dve", "recip", b - 1))
                ACT.op(lambda b=b: a.activation(
                    out=attn_sb[(b - 1) % 2][0:1, 0:512],
                    in_=acc_lo.ap()[0:1, :],
                    func=Copy, scale=rZ[(b - 1) % 2]))
                ACT.mark("cplo", b - 1)
                ACT.op(lambda b=b: a.activation(
                    out=attn_sb[(b - 1) % 2][0:1, 512:1024],
                    in_=acc_hi.ap()[0:1, :],
                    func=Copy, scale=rZ[(b - 1) % 2]))
                ACT.mark("cphi", b - 1)
        emit_exp(BPC - 1)
        for b in (BPC - 1,):
            ACT.wait(("pe", "e2", b))
            ACT.wait(("dve", "recip", b))
            ACT.op(lambda b=b: a.activation(
                out=attn_sb[b % 2][0:1, 0:512], in_=acc_lo.ap()[0:1, :],
                func=Copy, scale=rZ[b % 2]))
            ACT.mark("cplo", b)
            ACT.op(lambda b=b: a.activation(
                out=attn_sb[b % 2][0:1, 512:1024], in_=acc_hi.ap()[0:1, :],
                func=Copy, scale=rZ[b % 2]))
            ACT.mark("cphi", b)
        for b in (BPC - 2, BPC - 1):
            ACT.wait(("pe", "attnT", b))
            ACT.op(lambda b=b: a.activation(
                out=CT.ap()[:, HCH:NCH, b], in_=ctcols, func=Copy))
            ACT.mark("ctcp", b)

    progs = [
        (GPS, prog_gps), (DMA, prog_dma), (PE, prog_pe),
        (DVE, prog_dve), (ACT, prog_act),
    ]

    # pass 1: count
    for pr, fn in progs:
        pr.begin(emit=False)
        fn()

    # pass 2: emit
    counts.clear()
    sem_names = ["pe", "dve", "act", "gps", "hid", "hbc", "bias",
                 "l0", "l1", "l2", "wt", "outd"]
    with nc.Block() as block:
        for sn in sem_names:
            sems[sn] = nc.alloc_semaphore(name=f"{sn}_sem")

        @block.gpsimd
        def _(eng):
            GPS.begin(eng=eng, emit=True)
            prog_gps()

        @block.sync
        def _(eng):
            DMA.begin(eng=eng, emit=True)
            prog_dma()

        @block.tensor
        def _(eng):
            PE.begin(eng=eng, emit=True)
            prog_pe()

        @block.vector
        def _(eng):
            DVE.begin(eng=eng, emit=True)
            prog_dve()

        @block.scalar
        def _(eng):
            ACT.begin(eng=eng, emit=True)
            prog_act()

    return nc


def kernel(lstm_output, hidden, W_combine, b_combine):
    global _cached_nc, last_results
    lstm_output = np.asarray(lstm_output, dtype=np.float32)
    hidden = np.asarray(hidden, dtype=np.float32)
    W_combine = np.asarray(W_combine, dtype=np.float32)
    b_combine = np.asarray(b_combine, dtype=np.float32)

    if _cached_nc is None:
        _cached_nc = _build_program()
    nc = _cached_nc

    # fp16 partition-major relayout: [B, S, H] -> [B, 128, T, H]
    l16 = lstm_output.astype(np.float16).reshape(B, T, 128, H).transpose(0, 2, 1, 3)
    wt16 = np.ascontiguousarray(
        W_combine.T.astype(np.float16).reshape(NCH, 128, H).transpose(1, 0, 2))
    hid16 = hidden.astype(np.float16)

    in_maps = []
    for i in range(NCORES):
        sl = slice(i * BPC, (i + 1) * BPC)
        in_maps.append({
            "lstm16": np.ascontiguousarray(l16[sl]),
            "hidden": np.ascontiguousarray(hidden[sl]),
            "hidden16": np.ascontiguousarray(hid16[sl]),
            "w_t16": wt16,
            "b_combine": b_combine,
        })
    res = run_bass_kernel_spmd(nc, in_maps, core_ids=list(range(NCORES)))
    last_results = res
    return np.concatenate([res.results[i]["out"] for i in range(NCORES)], axis=0)
